# revision 9
# baseline (speedup 1.0000x reference)
"""GQA attention block (nn_Attention_2851858284851) on 8 TRN2 NeuronCores.

Sequence-parallel sharding: core c owns query blocks {c, 15-c} (128 tokens
each) so causal work is balanced across cores. Per core:
  - project q/k/v for its 256 tokens (all heads), RMSNorm + RoPE
  - all-gather K^T and V (bf16, ~256KB/rank each) across the 8 cores
  - causal attention for its 2 query blocks over all 32 heads
  - o-projection for its 256 rows (contraction over all 4096 head-dims is
    fully local -> no output collective; host concatenates rows)

SPMD: all 8 cores execute one identical instruction stream. Per-core causal
structure is encoded in input data (mask tensors), never in loop bounds or
addresses. Compute dtype bf16 (fp32 PSUM accumulation), softmax in fp32;
RMSNorm bounds |scores| <= sqrt(D) so exp needs no max-subtraction.

Scores are computed transposed S.T[k, q] so the exp pass (ACT, PSUM->SBUF)
lands P^T directly where the PV matmul wants it; a ones-column appended to V
makes the PV matmul also produce the softmax row-sums.
"""

import sys

if "/opt/trn_rl_repo" not in sys.path:
    sys.path.insert(0, "/opt/trn_rl_repo")

import numpy as np
import ml_dtypes

BF16 = ml_dtypes.bfloat16

L, HID, D, H, HKV = 2048, 2048, 128, 32, 4
EPS = 1e-6
NC_ = 8
BLK = 128
NBLK = L // BLK   # 16
TPC = 2 * BLK     # tokens per core
NI = HID // 128   # 16 contraction chunks
GQ = H // HKV     # 8 q heads per kv head
ISCALE = float(1.0 / np.sqrt(D))


def core_blocks(c):
    return (c, NBLK - 1 - c)


def tok_rows(c):
    lo, hi = core_blocks(c)
    return np.r_[lo * BLK:(lo + 1) * BLK, hi * BLK:(hi + 1) * BLK]


def gpos(l):
    """Gathered (rank-major) [rank, slot] position of logical block l."""
    if l < NBLK // 2:
        return l, 0
    return NBLK - 1 - l, 1


def gcol(l):
    r, s = gpos(l)
    return 256 * r + 128 * s


def build_masks(c):
    """Causal masks (bf16 {0,1}) for transposed scores S.T[k, q].

    mAB [128, 2*512]: lo-query halves of key chunks 0-7 (hi halves of those
    chunks are always fully valid -> never multiplied).
    mC  [128, 8*128]: hi-query columns of key chunks 8-15.
    """
    lo, hi = core_blocks(c)
    tri = (np.arange(128)[None, :] >= np.arange(128)[:, None])
    ones = np.ones((128, 128), bool)
    zeros = np.zeros((128, 128), bool)

    def blk_mask(l, b):
        if l < b:
            return ones
        if l == b:
            return tri
        return zeros

    mAB = np.concatenate([blk_mask(l, lo) for l in range(8)], axis=1)
    mC = np.concatenate([blk_mask(l, hi) for l in range(8, 16)], axis=1)
    return mAB.astype(BF16), mC.astype(BF16)


def fold_rope(cos, sin, w):
    """Fold the RMSNorm weight into the RoPE tables:
    out[:, i] = qhat[:, i]*CW[:, i] + qhat[:, rot(i)]*SW[:, i], with
    rot(i) = i+64 (sign -) for i < 64, i-64 (sign +) otherwise."""
    half = D // 2
    cw = cos * w[None, :]
    sw = np.empty_like(sin)
    sw[:, :half] = -sin[:, :half] * w[None, half:]
    sw[:, half:] = sin[:, half:] * w[None, :half]
    return cw.astype(np.float32), sw.astype(np.float32)


# ---------------------------------------------------------------------------
# device graph
# ---------------------------------------------------------------------------
_COMPILED = None


def build():
    import concourse.tile as tile
    from concourse import bacc, mybir
    from concourse.masks import make_identity
    from contextlib import ExitStack

    dt = mybir.dt
    MUL = mybir.AluOpType.mult
    nc = bacc.Bacc("TRN2", target_bir_lowering=False, debug=False)

    xT_e = nc.declare_dram_parameter("xT", [HID, TPC], dt.bfloat16, isOutput=False)
    wqT_e = nc.declare_dram_parameter("wqT", [HID, H * D], dt.bfloat16, isOutput=False)
    wkvT_e = nc.declare_dram_parameter("wkvT", [HID, 2 * HKV * D], dt.bfloat16, isOutput=False)
    woT_e = nc.declare_dram_parameter("woT", [H * D, HID], dt.bfloat16, isOutput=False)
    cwq_e = nc.declare_dram_parameter("cwq", [TPC, D], dt.float32, isOutput=False)
    swq_e = nc.declare_dram_parameter("swq", [TPC, D], dt.float32, isOutput=False)
    cwk_e = nc.declare_dram_parameter("cwk", [TPC, D], dt.float32, isOutput=False)
    swk_e = nc.declare_dram_parameter("swk", [TPC, D], dt.float32, isOutput=False)
    mAB_e = nc.declare_dram_parameter("mAB", [128, 1024], dt.bfloat16, isOutput=False)
    mC_e = nc.declare_dram_parameter("mC", [128, 1024], dt.bfloat16, isOutput=False)
    out_e = nc.declare_dram_parameter("out", [TPC, HID], dt.float32, isOutput=True)

    kT_in = nc.dram_tensor("kT_in", [HKV * D, TPC], dt.bfloat16)
    kT_out = nc.dram_tensor("kT_out", [NC_ * HKV * D, TPC], dt.bfloat16, addr_space="Shared")
    v_in = nc.dram_tensor("v_in", [TPC, HKV * D], dt.bfloat16)
    v_out = nc.dram_tensor("v_out", [NC_ * TPC, HKV * D], dt.bfloat16, addr_space="Shared")

    rg = [list(range(NC_))]

    with tile.TileContext(nc) as tc, ExitStack() as ctx:
        pers = ctx.enter_context(tc.tile_pool(name="pers", bufs=1))
        wq_pool = ctx.enter_context(tc.tile_pool(name="wq", bufs=20))
        wo_pool = ctx.enter_context(tc.tile_pool(name="wo", bufs=10))
        small = ctx.enter_context(tc.tile_pool(name="small", bufs=6))
        stat = ctx.enter_context(tc.tile_pool(name="stat", bufs=8))
        ptp = ctx.enter_context(tc.tile_pool(name="pt", bufs=8))
        pp_proj = ctx.enter_context(tc.tile_pool(name="pp_proj", bufs=2, space="PSUM"))
        pp_pv = ctx.enter_context(tc.tile_pool(name="pp_pv", bufs=2, space="PSUM"))

        ident = pers.tile([128, 128], dt.bfloat16, tag="ident")
        make_identity(nc, ident[:])
        eps_t = pers.tile([128, 1], dt.float32, tag="eps")
        nc.vector.memset(eps_t[:], EPS)

        xT = pers.tile([128, NI, TPC], dt.bfloat16, tag="xT")
        for i in range(NI):
            nc.sync.dma_start(xT[:, i, :], xT_e[i * 128:(i + 1) * 128, :])

        wkv = pers.tile([128, NI, 2 * HKV * D], dt.bfloat16, tag="wkv")
        for i in range(NI):
            nc.sync.dma_start(wkv[:, i, :], wkvT_e[i * 128:(i + 1) * 128, :])

        cwq = pers.tile([128, 2, D], dt.float32, tag="cwq")
        swq = pers.tile([128, 2, D], dt.float32, tag="swq")
        cwk = pers.tile([128, 2, D], dt.float32, tag="cwk")
        swk = pers.tile([128, 2, D], dt.float32, tag="swk")
        for t, e in ((cwq, cwq_e), (swq, swq_e), (cwk, cwk_e), (swk, swk_e)):
            for tb in range(2):
                nc.sync.dma_start(t[:, tb, :], e[tb * 128:(tb + 1) * 128, :])

        mAB = pers.tile([128, 2, 4, 128], dt.bfloat16, tag="mAB")
        nc.sync.dma_start(mAB[:], mAB_e[:])
        mC = pers.tile([128, 1024], dt.bfloat16, tag="mC")
        nc.sync.dma_start(mC[:], mC_e[:])

        qT = pers.tile([128, H, TPC], dt.bfloat16, tag="qT")
        kT = pers.tile([128, HKV, L], dt.bfloat16, tag="kT")  # gathered col order
        v_aug = pers.tile([128, HKV, NBLK, D + 1], dt.bfloat16, tag="vaug")
        nc.vector.memset(v_aug[:], 1.0)  # ones column survives the v DMAs
        attnT = pers.tile([128, H, 2, 128], dt.bfloat16, tag="attnT")
        o_sb = pers.tile([128, 2, HID], dt.float32, tag="o_sb")

        def rmsnorm_rope(psum_slice, cw, sw, out_bf):
            """[128t, 128d] fp32 PSUM -> normed+roped bf16 SBUF."""
            sq = small.tile([128, 128], dt.float32, tag="sq")
            ssq = stat.tile([128, 1], dt.float32, tag="ssq")
            nc.scalar.activation(sq[:], psum_slice,
                                 mybir.ActivationFunctionType.Square,
                                 accum_out=ssq[:])
            std = stat.tile([128, 1], dt.float32, tag="std")
            nc.scalar.activation(std[:], ssq[:], mybir.ActivationFunctionType.Sqrt,
                                 bias=eps_t[:], scale=1.0 / D)
            rstd = stat.tile([128, 1], dt.float32, tag="rstd")
            nc.vector.reciprocal(rstd[:], std[:])
            half = D // 2
            a = small.tile([128, 128], dt.float32, tag="ra")
            m = small.tile([128, 128], dt.float32, tag="rm")
            nc.vector.scalar_tensor_tensor(out=a[:], in0=psum_slice, scalar=rstd[:],
                                           in1=cw, op0=MUL, op1=MUL)
            nc.vector.scalar_tensor_tensor(out=m[:, :half], in0=psum_slice[:, half:],
                                           scalar=rstd[:], in1=sw[:, :half],
                                           op0=MUL, op1=MUL)
            nc.vector.scalar_tensor_tensor(out=m[:, half:], in0=psum_slice[:, :half],
                                           scalar=rstd[:], in1=sw[:, half:],
                                           op0=MUL, op1=MUL)
            nc.vector.tensor_add(out_bf, a[:], m[:])

        # ================= K/V projection for own 256 tokens ================
        for tb in range(2):
            k_ps = pp_proj.tile([128, 512], dt.float32, tag="proj")
            v_ps = pp_proj.tile([128, 512], dt.float32, tag="proj")
            for i in range(NI):
                nc.tensor.matmul(k_ps[:], xT[:, i, tb * 128:(tb + 1) * 128],
                                 wkv[:, i, 0:512], start=(i == 0), stop=(i == NI - 1))
            for i in range(NI):
                nc.tensor.matmul(v_ps[:], xT[:, i, tb * 128:(tb + 1) * 128],
                                 wkv[:, i, 512:1024], start=(i == 0), stop=(i == NI - 1))
            for h in range(HKV):
                kbf = small.tile([128, 128], dt.bfloat16, tag="kbf")
                rmsnorm_rope(k_ps[:, h * 128:(h + 1) * 128],
                             cwk[:, tb, :], swk[:, tb, :], kbf[:])
                ktp = pp_pv.tile([128, 128], dt.bfloat16, tag="pv")
                nc.tensor.transpose(ktp[:], kbf[:], ident[:])
                kts = small.tile([128, 128], dt.bfloat16, tag="kts")
                nc.vector.tensor_copy(kts[:], ktp[:])
                nc.sync.dma_start(
                    kT_in[h * 128:(h + 1) * 128, tb * 128:(tb + 1) * 128], kts[:])
            vbf = small.tile([128, 512], dt.bfloat16, tag="vbf")
            nc.vector.tensor_copy(vbf[:], v_ps[:])
            nc.sync.dma_start(v_in[tb * 128:(tb + 1) * 128, :], vbf[:])

        # ================= all-gather K^T and V (bf16) ======================
        nc.gpsimd.collective_compute("AllGather", mybir.AluOpType.bypass,
                                     replica_groups=rg, ins=[kT_in.ap().opt()],
                                     outs=[kT_out.ap().opt()])
        nc.gpsimd.collective_compute("AllGather", mybir.AluOpType.bypass,
                                     replica_groups=rg, ins=[v_in.ap().opt()],
                                     outs=[v_out.ap().opt()])

        # ---- assemble gathered K^T / V into SBUF (fires as AG lands) ----
        # kT keeps the gathered rank-major column order; scores index gcol(l).
        for h in range(HKV):
            for r in range(NC_):
                nc.sync.dma_start(
                    kT[:, h, r * 256:(r + 1) * 256],
                    kT_out[r * HKV * D + h * 128: r * HKV * D + (h + 1) * 128, :])
        for h in range(HKV):
            for l in range(NBLK):
                r, slot = gpos(l)
                nc.sync.dma_start(
                    v_aug[:, h, l, 0:D],
                    v_out[r * TPC + slot * 128: r * TPC + (slot + 1) * 128,
                          h * 128:(h + 1) * 128])

        # ============ interleaved Q projection + attention per kv head ======
        def qproj(jc):
            wts = []
            for i in range(NI):
                w = wq_pool.tile([128, 512], dt.bfloat16, tag="wq", name="wq")
                nc.sync.dma_start(w[:], wqT_e[i * 128:(i + 1) * 128,
                                              jc * 512:(jc + 1) * 512])
                wts.append(w)
            for tb in range(2):
                q_ps = pp_proj.tile([128, 512], dt.float32, tag="proj", name="q_ps")
                for i in range(NI):
                    nc.tensor.matmul(q_ps[:], xT[:, i, tb * 128:(tb + 1) * 128],
                                     wts[i][:], start=(i == 0), stop=(i == NI - 1))
                for hh in range(4):
                    h = jc * 4 + hh
                    qbf = small.tile([128, 128], dt.bfloat16, tag="qbf", name="qbf")
                    rmsnorm_rope(q_ps[:, hh * 128:(hh + 1) * 128],
                                 cwq[:, tb, :], swq[:, tb, :], qbf[:])
                    qtp = pp_pv.tile([128, 128], dt.bfloat16, tag="pv", name="qtp")
                    nc.tensor.transpose(qtp[:], qbf[:], ident[:])
                    nc.vector.tensor_copy(qT[:, h, tb * 128:(tb + 1) * 128], qtp[:])

        def attention(kh, pp_sc):
            for sub in range(GQ):
                h = kh * GQ + sub
                pts = []
                for g in range(3):
                    sc = pp_sc.tile([128, 1024], dt.float32, tag="sc", name="sc")
                    if g < 2:
                        for dc in range(4):
                            l = g * 4 + dc
                            nc.tensor.matmul(
                                sc[:, dc * 256:(dc + 1) * 256],
                                kT[:, kh, gcol(l):gcol(l) + 128],
                                qT[:, h, :], start=True, stop=True)
                    else:
                        for dc in range(8):
                            l = 8 + dc
                            nc.tensor.matmul(
                                sc[:, dc * 128:(dc + 1) * 128],
                                kT[:, kh, gcol(l):gcol(l) + 128],
                                qT[:, h, 128:256], start=True, stop=True)
                    pt = ptp.tile([128, 1024], dt.bfloat16, tag="pt", name="pt")
                    nc.scalar.activation(pt[:], sc[:],
                                         mybir.ActivationFunctionType.Exp,
                                         scale=ISCALE)
                    if g < 2:
                        # mask only the lo-query halves (hi halves always valid)
                        lo_view = pt[:].rearrange("p (c q) -> p c q", c=4)[:, :, 0:128]
                        nc.vector.tensor_tensor(lo_view, lo_view, mAB[:, g, :, :], MUL)
                        pts.append(pt)
                    else:
                        pt2 = ptp.tile([128, 1024], dt.bfloat16, tag="pt", name="pt2")
                        nc.vector.tensor_tensor(pt2[:], pt[:], mC[:], MUL)
                        pts.append(pt2)
                pv = [pp_pv.tile([128, D + 1], dt.float32, tag="pv", name="pv")
                      for _ in range(2)]
                for l in range(8):
                    g, dc = divmod(l, 4)
                    nc.tensor.matmul(pv[0][:], pts[g][:, dc * 256:dc * 256 + 128],
                                     v_aug[:, kh, l, :],
                                     start=(l == 0), stop=(l == 7))
                for l in range(NBLK):
                    if l < 8:
                        g, dc = divmod(l, 4)
                        lhs = pts[g][:, dc * 256 + 128:dc * 256 + 256]
                    else:
                        lhs = pts[2][:, (l - 8) * 128:(l - 7) * 128]
                    nc.tensor.matmul(pv[1][:], lhs, v_aug[:, kh, l, :],
                                     start=(l == 0), stop=(l == NBLK - 1))
                for slot in range(2):
                    r_ = stat.tile([128, 1], dt.float32, tag="recip", name="r_")
                    nc.vector.reciprocal(r_[:], pv[slot][:, D:D + 1])
                    abf = small.tile([128, 128], dt.bfloat16, tag="abf", name="abf")
                    nc.vector.tensor_scalar_mul(abf[:], pv[slot][:, 0:D], r_[:])
                    atp = pp_pv.tile([128, 128], dt.bfloat16, tag="pv", name="atp")
                    nc.tensor.transpose(atp[:], abf[:], ident[:])
                    nc.vector.tensor_copy(attnT[:, h, slot, :], atp[:])

        with tc.tile_pool(name="pp_sc", bufs=2, space="PSUM") as pp_sc:
            for kh in range(HKV):
                qproj(2 * kh)
                qproj(2 * kh + 1)
                attention(kh, pp_sc)

        # ================= o-projection =====================================
        with tc.tile_pool(name="pp_o", bufs=4, space="PSUM") as pp_o:
            for mh in range(2):
                acc = [[pp_o.tile([128, 512], dt.float32, tag="o", name="oacc")
                        for _ in range(2)] for _ in range(2)]
                for j in range(H):
                    wo_t = wo_pool.tile([128, 1024], dt.bfloat16, tag="wo", name="wo")
                    nc.sync.dma_start(wo_t[:], woT_e[j * 128:(j + 1) * 128,
                                                     mh * 1024:(mh + 1) * 1024])
                    for tb in range(2):
                        for mm in range(2):
                            nc.tensor.matmul(acc[tb][mm][:], attnT[:, j, tb, :],
                                             wo_t[:, mm * 512:(mm + 1) * 512],
                                             start=(j == 0), stop=(j == H - 1))
                for tb in range(2):
                    for mm in range(2):
                        nc.vector.tensor_copy(
                            o_sb[:, tb, mh * 1024 + mm * 512:
                                 mh * 1024 + (mm + 1) * 512],
                            acc[tb][mm][:])
        for tb in range(2):
            nc.sync.dma_start(out_e[tb * 128:(tb + 1) * 128, :], o_sb[:, tb, :])

    nc.compile()
    return nc


# ---------------------------------------------------------------------------
# host wrapper
# ---------------------------------------------------------------------------

def _prep_inputs(x, wq, wk, wv, wo, q_norm_w, k_norm_w, cos, sin):
    x2 = np.asarray(x, np.float32).reshape(L, HID)
    cos2 = np.asarray(cos, np.float32).reshape(L, D)
    sin2 = np.asarray(sin, np.float32).reshape(L, D)
    xT = np.ascontiguousarray(x2.T).astype(BF16)
    wqT = np.ascontiguousarray(np.asarray(wq, np.float32).T).astype(BF16)
    wkT = np.asarray(wk, np.float32).T
    wvT = np.asarray(wv, np.float32).T
    wkvT = np.ascontiguousarray(np.concatenate([wkT, wvT], axis=1)).astype(BF16)
    woT = np.ascontiguousarray(np.asarray(wo, np.float32).T).astype(BF16)
    cwq_f, swq_f = fold_rope(cos2, sin2, np.asarray(q_norm_w, np.float32))
    cwk_f, swk_f = fold_rope(cos2, sin2, np.asarray(k_norm_w, np.float32))

    in_maps = []
    for c in range(NC_):
        rows = tok_rows(c)
        mAB, mC = build_masks(c)
        in_maps.append({
            "xT": np.ascontiguousarray(xT[:, rows]),
            "wqT": wqT, "wkvT": wkvT, "woT": woT,
            "cwq": np.ascontiguousarray(cwq_f[rows]),
            "swq": np.ascontiguousarray(swq_f[rows]),
            "cwk": np.ascontiguousarray(cwk_f[rows]),
            "swk": np.ascontiguousarray(swk_f[rows]),
            "mAB": mAB, "mC": mC,
        })
    return in_maps


def run(inputs, trace=False, repeat=2):
    global _COMPILED
    from concourse.bass_utils import run_bass_kernel_spmd

    if _COMPILED is None:
        _COMPILED = build()
    in_maps = _prep_inputs(**inputs)
    res = None
    for _ in range(max(1, repeat)):
        res = run_bass_kernel_spmd(_COMPILED, in_maps, core_ids=list(range(NC_)),
                                   trace=trace)
    out = np.empty((L, HID), np.float32)
    for c in range(NC_):
        out[tok_rows(c)] = res.results[c]["out"]
    return out.reshape(1, L, HID), res


def kernel(x, wq, wk, wv, wo, q_norm_w, k_norm_w, cos, sin):
    out, _ = run(dict(x=x, wq=wq, wk=wk, wv=wv, wo=wo, q_norm_w=q_norm_w,
                      k_norm_w=k_norm_w, cos=cos, sin=sin), trace=False)
    return out


# revision 10
# speedup vs baseline: 1.1687x; 1.1687x over previous
"""GQA attention block (nn_Attention_2851858284851) on 8 TRN2 NeuronCores.

Sequence-parallel sharding: core c owns query blocks {c, 15-c} (128 tokens
each) so causal work is balanced across cores. Per core:
  - project q/k/v for its 256 tokens (all heads), RMSNorm + RoPE
  - all-gather K^T and V (bf16, ~256KB/rank each) across the 8 cores
  - causal attention for its 2 query blocks over all 32 heads
  - o-projection for its 256 rows (contraction over all 4096 head-dims is
    fully local -> no output collective; host concatenates rows)

SPMD: all 8 cores execute one identical instruction stream. Per-core causal
structure is encoded in input data (mask tensors), never in loop bounds or
addresses. Compute dtype bf16 (fp32 PSUM accumulation), softmax in fp32;
RMSNorm bounds |scores| <= sqrt(D) so exp needs no max-subtraction.

Scores are computed transposed S.T[k, q] so the exp pass (ACT, PSUM->SBUF)
lands P^T directly where the PV matmul wants it; a ones-column appended to V
makes the PV matmul also produce the softmax row-sums. Phases are kept
separate (proj | attention | o-proj) so the ACT function table is stable
within each phase (Square/Sqrt vs Exp swaps cost ~1.3us each).
"""

import sys

if "/opt/trn_rl_repo" not in sys.path:
    sys.path.insert(0, "/opt/trn_rl_repo")

import numpy as np
import ml_dtypes

BF16 = ml_dtypes.bfloat16

L, HID, D, H, HKV = 2048, 2048, 128, 32, 4
EPS = 1e-6
NC_ = 8
BLK = 128
NBLK = L // BLK   # 16
TPC = 2 * BLK     # tokens per core
NI = HID // 128   # 16 contraction chunks
GQ = H // HKV     # 8 q heads per kv head
ISCALE = float(1.0 / np.sqrt(D))


def core_blocks(c):
    return (c, NBLK - 1 - c)


def tok_rows(c):
    lo, hi = core_blocks(c)
    return np.r_[lo * BLK:(lo + 1) * BLK, hi * BLK:(hi + 1) * BLK]


def gpos(l):
    """Gathered (rank-major) [rank, slot] position of logical block l."""
    if l < NBLK // 2:
        return l, 0
    return NBLK - 1 - l, 1


def gcol(l):
    r, s = gpos(l)
    return 256 * r + 128 * s


def build_masks(c):
    """Causal masks (bf16 {0,1}) for transposed scores S.T[k, q].

    mAB [128, 8*128]: lo-query halves of key chunks 0-7 (hi halves of those
    chunks are always fully valid -> never multiplied).
    mC  [128, 8*128]: hi-query columns of key chunks 8-15.
    """
    lo, hi = core_blocks(c)
    tri = (np.arange(128)[None, :] >= np.arange(128)[:, None])
    ones = np.ones((128, 128), bool)
    zeros = np.zeros((128, 128), bool)

    def blk_mask(l, b):
        if l < b:
            return ones
        if l == b:
            return tri
        return zeros

    mAB = np.concatenate([blk_mask(l, lo) for l in range(8)], axis=1)
    mC = np.concatenate([blk_mask(l, hi) for l in range(8, 16)], axis=1)
    return mAB.astype(BF16), mC.astype(BF16)


def fold_rope(cos, sin, w):
    """Fold the RMSNorm weight into the RoPE tables:
    out[:, i] = qhat[:, i]*CW[:, i] + qhat[:, rot(i)]*SW[:, i], with
    rot(i) = i+64 (sign -) for i < 64, i-64 (sign +) otherwise."""
    half = D // 2
    cw = cos * w[None, :]
    sw = np.empty_like(sin)
    sw[:, :half] = -sin[:, :half] * w[None, half:]
    sw[:, half:] = sin[:, half:] * w[None, :half]
    return cw.astype(np.float32), sw.astype(np.float32)


# ---------------------------------------------------------------------------
# device graph
# ---------------------------------------------------------------------------
_COMPILED = None


def build():
    import concourse.tile as tile
    from concourse import bacc, mybir
    from concourse.masks import make_identity
    from contextlib import ExitStack

    dt = mybir.dt
    MUL = mybir.AluOpType.mult
    nc = bacc.Bacc("TRN2", target_bir_lowering=False, debug=False)

    xT_e = nc.declare_dram_parameter("xT", [HID, TPC], dt.bfloat16, isOutput=False)
    wqT_e = nc.declare_dram_parameter("wqT", [HID, H * D], dt.bfloat16, isOutput=False)
    wkvT_e = nc.declare_dram_parameter("wkvT", [HID, 2 * HKV * D], dt.bfloat16, isOutput=False)
    woT_e = nc.declare_dram_parameter("woT", [H * D, HID], dt.bfloat16, isOutput=False)
    cwq_e = nc.declare_dram_parameter("cwq", [TPC, D], dt.float32, isOutput=False)
    swq_e = nc.declare_dram_parameter("swq", [TPC, D], dt.float32, isOutput=False)
    cwk_e = nc.declare_dram_parameter("cwk", [TPC, D], dt.float32, isOutput=False)
    swk_e = nc.declare_dram_parameter("swk", [TPC, D], dt.float32, isOutput=False)
    mAB_e = nc.declare_dram_parameter("mAB", [128, 1024], dt.bfloat16, isOutput=False)
    mC_e = nc.declare_dram_parameter("mC", [128, 1024], dt.bfloat16, isOutput=False)
    out_e = nc.declare_dram_parameter("out", [TPC, HID], dt.float32, isOutput=True)

    kT_in = nc.dram_tensor("kT_in", [HKV * D, TPC], dt.bfloat16)
    kT_out = nc.dram_tensor("kT_out", [NC_ * HKV * D, TPC], dt.bfloat16, addr_space="Shared")
    v_in = nc.dram_tensor("v_in", [TPC, HKV * D], dt.bfloat16)
    v_out = nc.dram_tensor("v_out", [NC_ * TPC, HKV * D], dt.bfloat16, addr_space="Shared")

    rg = [list(range(NC_))]

    with tile.TileContext(nc) as tc, ExitStack() as ctx:
        pers = ctx.enter_context(tc.tile_pool(name="pers", bufs=1))
        wq_pool = ctx.enter_context(tc.tile_pool(name="wq", bufs=24))
        wo_pool = ctx.enter_context(tc.tile_pool(name="wo", bufs=8))
        small = ctx.enter_context(tc.tile_pool(name="small", bufs=6))
        stat = ctx.enter_context(tc.tile_pool(name="stat", bufs=8))
        ptp = ctx.enter_context(tc.tile_pool(name="pt", bufs=6))
        pp_pv = ctx.enter_context(tc.tile_pool(name="pp_pv", bufs=2, space="PSUM"))

        ident = pers.tile([128, 128], dt.bfloat16, tag="ident")
        make_identity(nc, ident[:])
        eps_t = pers.tile([128, 1], dt.float32, tag="eps")
        nc.vector.memset(eps_t[:], EPS)

        xT = pers.tile([128, NI, TPC], dt.bfloat16, tag="xT")
        for i in range(NI):
            nc.sync.dma_start(xT[:, i, :], xT_e[i * 128:(i + 1) * 128, :])

        wkv = pers.tile([128, NI, 2 * HKV * D], dt.bfloat16, tag="wkv")
        for i in range(NI):
            nc.sync.dma_start(wkv[:, i, :], wkvT_e[i * 128:(i + 1) * 128, :])

        cwq = pers.tile([128, 2, D], dt.float32, tag="cwq")
        swq = pers.tile([128, 2, D], dt.float32, tag="swq")
        cwk = pers.tile([128, 2, D], dt.float32, tag="cwk")
        swk = pers.tile([128, 2, D], dt.float32, tag="swk")
        for t, e in ((cwq, cwq_e), (swq, swq_e), (cwk, cwk_e), (swk, swk_e)):
            for tb in range(2):
                nc.sync.dma_start(t[:, tb, :], e[tb * 128:(tb + 1) * 128, :])

        mAB = pers.tile([128, 2, 4, 128], dt.bfloat16, tag="mAB")
        nc.sync.dma_start(mAB[:], mAB_e[:])
        mC = pers.tile([128, 1024], dt.bfloat16, tag="mC")
        nc.sync.dma_start(mC[:], mC_e[:])

        qT = pers.tile([128, H, TPC], dt.bfloat16, tag="qT")
        kT = pers.tile([128, HKV, L], dt.bfloat16, tag="kT")  # gathered col order
        v_aug = pers.tile([128, HKV, NBLK, D + 1], dt.bfloat16, tag="vaug")
        nc.vector.memset(v_aug[:], 1.0)  # ones column survives the v DMAs
        attnT = pers.tile([128, H, 2, 128], dt.bfloat16, tag="attnT")

        def rmsnorm_rope(psum_slice, cw, sw, out_bf):
            """[128t, 128d] fp32 PSUM -> normed+roped bf16 SBUF."""
            sq = small.tile([128, 128], dt.float32, tag="sq", name="sq")
            ssq = stat.tile([128, 1], dt.float32, tag="ssq", name="ssq")
            nc.scalar.activation(sq[:], psum_slice,
                                 mybir.ActivationFunctionType.Square,
                                 accum_out=ssq[:])
            std = stat.tile([128, 1], dt.float32, tag="std", name="std")
            nc.scalar.activation(std[:], ssq[:], mybir.ActivationFunctionType.Sqrt,
                                 bias=eps_t[:], scale=1.0 / D)
            rstd = stat.tile([128, 1], dt.float32, tag="rstd", name="rstd")
            nc.vector.reciprocal(rstd[:], std[:])
            half = D // 2
            a = small.tile([128, 128], dt.float32, tag="ra", name="ra")
            m = small.tile([128, 128], dt.float32, tag="rm", name="rm")
            nc.vector.scalar_tensor_tensor(out=a[:], in0=psum_slice, scalar=rstd[:],
                                           in1=cw, op0=MUL, op1=MUL)
            nc.vector.scalar_tensor_tensor(out=m[:, :half], in0=psum_slice[:, half:],
                                           scalar=rstd[:], in1=sw[:, :half],
                                           op0=MUL, op1=MUL)
            nc.vector.scalar_tensor_tensor(out=m[:, half:], in0=psum_slice[:, :half],
                                           scalar=rstd[:], in1=sw[:, half:],
                                           op0=MUL, op1=MUL)
            nc.vector.tensor_add(out_bf, a[:], m[:])

        # ================= projections (K/V then Q) =========================
        with tc.tile_pool(name="pp_proj", bufs=2, space="PSUM") as pp_proj:
            for tb in range(2):
                k_ps = pp_proj.tile([128, 512], dt.float32, tag="proj", name="k_ps")
                v_ps = pp_proj.tile([128, 512], dt.float32, tag="proj", name="v_ps")
                for i in range(NI):
                    nc.tensor.matmul(k_ps[:], xT[:, i, tb * 128:(tb + 1) * 128],
                                     wkv[:, i, 0:512], start=(i == 0), stop=(i == NI - 1))
                for i in range(NI):
                    nc.tensor.matmul(v_ps[:], xT[:, i, tb * 128:(tb + 1) * 128],
                                     wkv[:, i, 512:1024], start=(i == 0), stop=(i == NI - 1))
                for h in range(HKV):
                    kbf = small.tile([128, 128], dt.bfloat16, tag="kbf", name="kbf")
                    rmsnorm_rope(k_ps[:, h * 128:(h + 1) * 128],
                                 cwk[:, tb, :], swk[:, tb, :], kbf[:])
                    ktp = pp_pv.tile([128, 128], dt.bfloat16, tag="pv", name="ktp")
                    nc.tensor.transpose(ktp[:], kbf[:], ident[:])
                    kts = small.tile([128, 128], dt.bfloat16, tag="kts", name="kts")
                    nc.vector.tensor_copy(kts[:], ktp[:])
                    nc.sync.dma_start(
                        kT_in[h * 128:(h + 1) * 128, tb * 128:(tb + 1) * 128], kts[:])
                vbf = small.tile([128, 512], dt.bfloat16, tag="vbf", name="vbf")
                nc.vector.tensor_copy(vbf[:], v_ps[:])
                nc.sync.dma_start(v_in[tb * 128:(tb + 1) * 128, :], vbf[:])

            # ---- all-gather K^T and V (bf16) ----
            nc.gpsimd.collective_compute("AllGather", mybir.AluOpType.bypass,
                                         replica_groups=rg, ins=[kT_in.ap().opt()],
                                         outs=[kT_out.ap().opt()])
            nc.gpsimd.collective_compute("AllGather", mybir.AluOpType.bypass,
                                         replica_groups=rg, ins=[v_in.ap().opt()],
                                         outs=[v_out.ap().opt()])

            # ---- Q projection (overlaps the AG; wq streams ahead of the
            #      post-AG assembly DMAs in the queues) ----
            for jc in range(8):
                wts = []
                for i in range(NI):
                    w = wq_pool.tile([128, 512], dt.bfloat16, tag="wq", name="wq")
                    nc.sync.dma_start(w[:], wqT_e[i * 128:(i + 1) * 128,
                                                  jc * 512:(jc + 1) * 512])
                    wts.append(w)
                for tb in range(2):
                    q_ps = pp_proj.tile([128, 512], dt.float32, tag="proj", name="q_ps")
                    for i in range(NI):
                        nc.tensor.matmul(q_ps[:], xT[:, i, tb * 128:(tb + 1) * 128],
                                         wts[i][:], start=(i == 0), stop=(i == NI - 1))
                    for hh in range(4):
                        h = jc * 4 + hh
                        qbf = small.tile([128, 128], dt.bfloat16, tag="qbf", name="qbf")
                        rmsnorm_rope(q_ps[:, hh * 128:(hh + 1) * 128],
                                     cwq[:, tb, :], swq[:, tb, :], qbf[:])
                        qtp = pp_pv.tile([128, 128], dt.bfloat16, tag="pv", name="qtp")
                        nc.tensor.transpose(qtp[:], qbf[:], ident[:])
                        nc.vector.tensor_copy(qT[:, h, tb * 128:(tb + 1) * 128], qtp[:])

        # ---- assemble gathered K^T / V into SBUF (queued after wq stream) --
        for h in range(HKV):
            for r in range(NC_):
                nc.sync.dma_start(
                    kT[:, h, r * 256:(r + 1) * 256],
                    kT_out[r * HKV * D + h * 128: r * HKV * D + (h + 1) * 128, :])
        for h in range(HKV):
            for l in range(NBLK):
                r, slot = gpos(l)
                nc.sync.dma_start(
                    v_aug[:, h, l, 0:D],
                    v_out[r * TPC + slot * 128: r * TPC + (slot + 1) * 128,
                          h * 128:(h + 1) * 128])

        # ================= attention ========================================
        with tc.tile_pool(name="pp_sc", bufs=3, space="PSUM") as pp_sc:
            for kh in range(HKV):
                for sub in range(GQ):
                    h = kh * GQ + sub
                    pts = []
                    for g in range(3):
                        sc = pp_sc.tile([128, 1024], dt.float32, tag="sc", name="sc")
                        if g < 2:
                            for dc in range(4):
                                l = g * 4 + dc
                                nc.tensor.matmul(
                                    sc[:, dc * 256:(dc + 1) * 256],
                                    kT[:, kh, gcol(l):gcol(l) + 128],
                                    qT[:, h, :], start=True, stop=True)
                        else:
                            for dc in range(8):
                                l = 8 + dc
                                nc.tensor.matmul(
                                    sc[:, dc * 128:(dc + 1) * 128],
                                    kT[:, kh, gcol(l):gcol(l) + 128],
                                    qT[:, h, 128:256], start=True, stop=True)
                        pt = ptp.tile([128, 1024], dt.bfloat16, tag="pt", name="pt")
                        nc.scalar.activation(pt[:], sc[:],
                                             mybir.ActivationFunctionType.Exp,
                                             scale=ISCALE)
                        if g < 2:
                            # mask only lo-query halves (hi halves always valid)
                            lo_view = pt[:].rearrange("p (c q) -> p c q", c=4)[:, :, 0:128]
                            nc.vector.tensor_tensor(lo_view, lo_view, mAB[:, g, :, :], MUL)
                            pts.append(pt)
                        else:
                            pt2 = ptp.tile([128, 1024], dt.bfloat16, tag="pt", name="pt2")
                            nc.vector.tensor_tensor(pt2[:], pt[:], mC[:], MUL)
                            pts.append(pt2)
                    pv = [pp_pv.tile([128, D + 1], dt.float32, tag="pv", name="pv")
                          for _ in range(2)]
                    for l in range(8):
                        g, dc = divmod(l, 4)
                        nc.tensor.matmul(pv[0][:], pts[g][:, dc * 256:dc * 256 + 128],
                                         v_aug[:, kh, l, :],
                                         start=(l == 0), stop=(l == 7))
                    for l in range(NBLK):
                        if l < 8:
                            g, dc = divmod(l, 4)
                            lhs = pts[g][:, dc * 256 + 128:dc * 256 + 256]
                        else:
                            lhs = pts[2][:, (l - 8) * 128:(l - 7) * 128]
                        nc.tensor.matmul(pv[1][:], lhs, v_aug[:, kh, l, :],
                                         start=(l == 0), stop=(l == NBLK - 1))
                    for slot in range(2):
                        r_ = stat.tile([128, 1], dt.float32, tag="recip", name="r_")
                        nc.vector.reciprocal(r_[:], pv[slot][:, D:D + 1])
                        abf = small.tile([128, 128], dt.bfloat16, tag="abf", name="abf")
                        nc.vector.tensor_scalar_mul(abf[:], pv[slot][:, 0:D], r_[:])
                        atp = pp_pv.tile([128, 128], dt.bfloat16, tag="pv", name="atp")
                        nc.tensor.transpose(atp[:], abf[:], ident[:])
                        nc.vector.tensor_copy(attnT[:, h, slot, :], atp[:])

        # ================= o-projection =====================================
        with tc.tile_pool(name="pp_o", bufs=4, space="PSUM") as pp_o:
            for mh in range(2):
                acc = [[pp_o.tile([128, 512], dt.float32, tag="o", name="oacc")
                        for _ in range(2)] for _ in range(2)]
                for j in range(H):
                    wo_t = wo_pool.tile([128, 1024], dt.bfloat16, tag="wo", name="wo")
                    nc.sync.dma_start(wo_t[:], woT_e[j * 128:(j + 1) * 128,
                                                     mh * 1024:(mh + 1) * 1024])
                    for tb in range(2):
                        for mm in range(2):
                            nc.tensor.matmul(acc[tb][mm][:], attnT[:, j, tb, :],
                                             wo_t[:, mm * 512:(mm + 1) * 512],
                                             start=(j == 0), stop=(j == H - 1))
                for tb in range(2):
                    for mm in range(2):
                        ost = small.tile([128, 512], dt.float32, tag="ost", name="ost")
                        nc.vector.tensor_copy(ost[:], acc[tb][mm][:])
                        nc.sync.dma_start(
                            out_e[tb * 128:(tb + 1) * 128,
                                  mh * 1024 + mm * 512: mh * 1024 + (mm + 1) * 512],
                            ost[:])

    nc.compile()
    return nc


# ---------------------------------------------------------------------------
# host wrapper
# ---------------------------------------------------------------------------

def _prep_inputs(x, wq, wk, wv, wo, q_norm_w, k_norm_w, cos, sin):
    x2 = np.asarray(x, np.float32).reshape(L, HID)
    cos2 = np.asarray(cos, np.float32).reshape(L, D)
    sin2 = np.asarray(sin, np.float32).reshape(L, D)
    xT = np.ascontiguousarray(x2.T).astype(BF16)
    wqT = np.ascontiguousarray(np.asarray(wq, np.float32).T).astype(BF16)
    wkT = np.asarray(wk, np.float32).T
    wvT = np.asarray(wv, np.float32).T
    wkvT = np.ascontiguousarray(np.concatenate([wkT, wvT], axis=1)).astype(BF16)
    woT = np.ascontiguousarray(np.asarray(wo, np.float32).T).astype(BF16)
    cwq_f, swq_f = fold_rope(cos2, sin2, np.asarray(q_norm_w, np.float32))
    cwk_f, swk_f = fold_rope(cos2, sin2, np.asarray(k_norm_w, np.float32))

    in_maps = []
    for c in range(NC_):
        rows = tok_rows(c)
        mAB, mC = build_masks(c)
        in_maps.append({
            "xT": np.ascontiguousarray(xT[:, rows]),
            "wqT": wqT, "wkvT": wkvT, "woT": woT,
            "cwq": np.ascontiguousarray(cwq_f[rows]),
            "swq": np.ascontiguousarray(swq_f[rows]),
            "cwk": np.ascontiguousarray(cwk_f[rows]),
            "swk": np.ascontiguousarray(swk_f[rows]),
            "mAB": mAB, "mC": mC,
        })
    return in_maps


def run(inputs, trace=False, repeat=2):
    global _COMPILED
    from concourse.bass_utils import run_bass_kernel_spmd

    if _COMPILED is None:
        _COMPILED = build()
    in_maps = _prep_inputs(**inputs)
    res = None
    for _ in range(max(1, repeat)):
        res = run_bass_kernel_spmd(_COMPILED, in_maps, core_ids=list(range(NC_)),
                                   trace=trace)
    out = np.empty((L, HID), np.float32)
    for c in range(NC_):
        out[tok_rows(c)] = res.results[c]["out"]
    return out.reshape(1, L, HID), res


def kernel(x, wq, wk, wv, wo, q_norm_w, k_norm_w, cos, sin):
    out, _ = run(dict(x=x, wq=wq, wk=wk, wv=wv, wo=wo, q_norm_w=q_norm_w,
                      k_norm_w=k_norm_w, cos=cos, sin=sin), trace=False)
    return out
